# revision 1
# baseline (speedup 1.0000x reference)
"""Trainium2 Bass kernel for nn_BlocksCore (moe_routing).

Contract: kernel(**inputs) takes FULL unsharded inputs (inp (4096,512),
hx/cx (4096,2048), weights, step) and returns (hx_out, cx_out, mask) each
(4096, 2048) f32, matching reference._fwd.

Strategy: pure data parallel over 8 NeuronCores (512 batch rows each).
Host precomputes Wcomb[k] = Wv_i[1] @ Wih[k].T (halves the dominant matmul
FLOPs and Wih HBM traffic), transposes activations for the PE, and casts
weight streams to bf16. The null-slot input attention collapses to a
sigmoid; the top-k freeze mask is a per-row threshold (4th largest score).
The score path is computed in true fp32 (reduced precision there flips
top-4 rankings).

Per-core program (Tile framework):
  phase S (per batch-chunk of 128): scores s, sig=sigmoid(s/8), top-4
    threshold mask (iterative max-removal), u8 + f32 expanded masks.
  phase G (k-outer, weights streamed once): gates = sig*(inp@Wcomb[k]) +
    hb@WhhT[k] (+bias) in PSUM; ACT nonlins; fused LSTM cell per (k, cb);
    cx_out blended in place via copy_predicated.
  phase A (per cb): h transpose, mha projections, per-sample 4-head
    attention on DVE via broadcast-AP products + grouped reduces, fc/gate
    residual gating, h_f = h_new + att, hx_out blend, DMA out.
"""
import os
import sys

import numpy as np

try:
    import concourse.bass as bass
except ImportError:  # container puts the repo here
    for _p in ("/opt/trn_rl_repo", "/root/.axon_site/_ro/trn_rl_repo"):
        if os.path.isdir(_p) and _p not in sys.path:
            sys.path.insert(0, _p)
    import concourse.bass as bass

import ml_dtypes
import concourse.bacc as bacc
import concourse.mybir as mybir
import concourse.tile as tile
from concourse.bass_utils import run_bass_kernel_spmd
from concourse.masks import make_identity

F32 = mybir.dt.float32
F32R = mybir.dt.float32r
BF16 = mybir.dt.bfloat16
F8 = mybir.dt.float8e4
U8 = mybir.dt.uint8
AF = mybir.ActivationFunctionType
ALU = mybir.AluOpType
AX = mybir.AxisListType
BF = ml_dtypes.bfloat16

PASSES_OVERRIDE = [[0, 1], [2], [3]]
NCORES = 8
P = 128          # partition rows per batch chunk
NK = 8           # blocks
HD = 256         # block size (BS)
GD = 1024        # gates per block (4*HD)
C = 512          # NINP
NH, DKM = 4, 16  # mha heads, head dim
EM = NH * DKM    # 64


def _build_program(bpc, has_bias, has_bias2):
    """Build the per-core Bass/Tile program. bpc = batch rows per core."""
    ncb = bpc // P
    nc = bacc.Bacc("TRN2", target_bir_lowering=False, debug=False,
                   num_devices=NCORES)

    din = {}
    def dram_in(name, shape, dtype=F32):
        din[name] = nc.dram_tensor(name, list(shape), dtype,
                                   kind="ExternalInput").ap()
        return din[name]

    hxT16d = dram_in("hxT16", (NK * HD, bpc), BF16)
    inp16b = dram_in("inp16b", (bpc, C), BF16)
    sig_d = dram_in("sig", (bpc, NK))
    mblk_d = dram_in("mblk", (bpc, NK))
    mblk8_d = dram_in("mblk8", (bpc, NK), U8)
    hx = dram_in("hx", (bpc, NK * HD))
    cx = dram_in("cx", (bpc, NK * HD))
    wcomb = dram_in("wcomb", (NK, C, GD), F8)
    whhT = dram_in("whhT", (NK, HD, GD), BF16)
    wmha = dram_in("wmha", (NK, HD, 3 * EM), BF16)
    wfg = dram_in("wfg", (EM, 2 * HD), BF16)
    if has_bias:
        biasg = dram_in("biasg", (NK, GD))
    if has_bias2:
        biasfg = dram_in("biasfg", (1, 2 * HD))

    hx_out = nc.dram_tensor("hx_out", [bpc, NK * HD], F32,
                            kind="ExternalOutput").ap()
    cx_out = nc.dram_tensor("cx_out", [bpc, NK * HD], F32,
                            kind="ExternalOutput").ap()
    mask_out = nc.dram_tensor("mask", [bpc, NK * HD], F32,
                              kind="ExternalOutput").ap()

    with tile.TileContext(nc) as tc:
        _emit(tc, din, hx_out, cx_out, mask_out, ncb, has_bias, has_bias2)
    nc.compile()
    return nc


def _emit(tc, din, hx_out, cx_out, mask_out, ncb, has_bias, has_bias2):
    nc = tc.nc
    bpc = ncb * P
    import contextlib
    ctx = tc._emit_ctx = __import__('contextlib').ExitStack()
    p1 = ctx.enter_context(tc.tile_pool(name="p1", bufs=1))
    p2 = ctx.enter_context(tc.tile_pool(name="p2", bufs=2))
    p3 = ctx.enter_context(tc.tile_pool(name="p3", bufs=3))
    psG = ctx.enter_context(tc.tile_pool(name="psG", bufs=2, space="PSUM"))
    psH = ctx.enter_context(tc.tile_pool(name="psH", bufs=2, space="PSUM"))
    psA = ctx.enter_context(tc.tile_pool(name="psA", bufs=2, space="PSUM"))
    psT = ctx.enter_context(tc.tile_pool(name="psT", bufs=2, space="PSUM"))

    # -------- static loads: tiny host-computed score/sig/mask inputs ------
    sig_all = p1.tile([P, ncb, NK], F32, tag="sig_all")
    nc.sync.dma_start(out=sig_all, in_=din["sig"].rearrange(
        "(cb p) k -> p cb k", p=P))
    mblk_all = p1.tile([P, ncb, NK], F32, tag="mblk_all")
    nc.sync.dma_start(out=mblk_all, in_=din["mblk"].rearrange(
        "(cb p) k -> p cb k", p=P))
    mblk8_all = p1.tile([P, ncb, NK], U8, tag="mblk8_all")
    nc.sync.dma_start(out=mblk8_all, in_=din["mblk8"].rearrange(
        "(cb p) k -> p cb k", p=P))
    wfg_t = p1.tile([EM, 2 * HD], BF16, tag="wfg")
    nc.sync.dma_start(out=wfg_t, in_=din["wfg"])
    identF = p1.tile([P, P], F32, tag="identF")
    make_identity(nc, identF)
    identB = p1.tile([P, P], BF16, tag="identB")
    nc.vector.tensor_copy(out=identB, in_=identF)
    if has_bias:
        biasg_t = p1.tile([1, NK, GD], F32, tag="biasg")
        nc.sync.dma_start(out=biasg_t, in_=din["biasg"].unsqueeze(0))
        onesF = p1.tile([1, P], F32, tag="onesF")
        nc.vector.memset(onesF, 1.0)
    if has_bias2:
        biasfg_t = p1.tile([1, 2 * HD], F32, tag="biasfg")
        nc.sync.dma_start(out=biasfg_t, in_=din["biasfg"])
        if not has_bias:
            onesF = p1.tile([1, P], F32, tag="onesF")
            nc.vector.memset(onesF, 1.0)

    sig_t = [sig_all[:, cb, :] for cb in range(ncb)]
    mblk_t = [mblk_all[:, cb, :] for cb in range(ncb)]
    masku8_t = []
    for cb in range(ncb):
        m8 = p1.tile([P, NK, HD], U8, tag=f"m8{cb}", name=f"m8_{cb}")
        nc.gpsimd.tensor_copy(out=m8, in_=mblk8_all[:, cb, :].unsqueeze(2)
                              .broadcast_to([P, NK, HD]))
        masku8_t.append(m8)

    # ---------------- deferred bulk loads (after phase-S DMAs) ----------------
    inp16b_t = p1.tile([P, ncb, 4, P], BF16, tag="inp16b")
    for cb in range(ncb):
        nc.sync.dma_start(out=inp16b_t[:, cb, :, :],
                          in_=din["inp16b"][cb * P:(cb + 1) * P, :]
                          .rearrange("p (c q) -> p c q", c=4))
    hxT16 = p1.tile([P, 16, bpc], BF16, tag="hxT16")
    hxT16_r = din["hxT16"].rearrange("(h p) b -> p h b", p=P)
    nc.sync.dma_start(out=hxT16[:, 0:2, :], in_=hxT16_r[:, 0:2, :])

    # ---------------- phase G: gates + fused LSTM cell ----------------
    wh_res = p1.tile([P, 4, 2, GD], BF16, tag="whres", name="wh_res")
    wmha_t = p1.tile([P, 2, NK, 3 * EM], BF16, tag="wmha", name="wmha_t")

    def load_wmha():
        for k in range(NK):
            nc.sync.dma_start(out=wmha_t[:, :, k, :],
                              in_=din["wmha"][k].rearrange(
                                  "(c p) e -> p c e", p=P))
    h_new = [p1.tile([P, NK * HD], BF16, tag=f"hnew{cb}", name=f"hnew{cb}")
             for cb in range(ncb)]

    def gates_lstm(k, cb, cxt, wc_h, wh_h):
        bsl = slice(cb * P, (cb + 1) * P)
        # inp_use^T = (inp_bmajor)^T @ diag(sig_k): stationary pre-scale
        diag = p3.tile([P, P], BF16, tag="diag", bufs=2, name=f"dg{k}_{cb}")
        nc.vector.tensor_scalar_mul(diag, in0=identB,
                                    scalar1=sig_t[cb][:, k:k + 1])
        dps = psG.tile([P, 4, P], F32, tag="dps", bufs=2, name=f"dp{k}_{cb}")
        for cc in range(4):
            nc.tensor.matmul(dps[:, cc, :], inp16b_t[:, cb, cc, :], diag,
                             start=True, stop=True)
        iuT = p3.tile([P, 4, P], F8, tag="iuT", bufs=2, name=f"iu{k}_{cb}")
        nc.scalar.copy(out=iuT, in_=dps)
        ifgo = p3.tile([P, 4, HD], BF16, tag="ifgo", bufs=4,
                       name=f"ifgo{k}_{cb}")
        for half in range(2):
            gsl = slice(half * 512, (half + 1) * 512)
            hh = psH.tile([P, 512], F32, tag="hh", bufs=3,
                          name=f"hh{k}_{cb}_{half}")
            for hc in range(2):
                nc.tensor.matmul(hh, hxT16[:, 2 * k + hc, bsl],
                                 wh_h[half][:, hc, :],
                                 start=(hc == 0), stop=False)
            if has_bias:
                nc.tensor.matmul(hh, onesF[0:1, 0:P].bitcast(F32R),
                                 biasg_t[0:1, k, gsl].bitcast(F32R),
                                 start=False, stop=False)
            for cc in range(4):
                nc.tensor.matmul(hh, iuT[:, cc, :], wc_h[half][:, cc, :],
                                 start=False, stop=(cc == 3))
            if half == 0:   # i, f
                nc.scalar.activation(out=ifgo[:, 0:2, :], in_=hh
                                     .rearrange("p (a e) -> p a e", a=2),
                                     func=AF.Sigmoid)
            else:           # g (tanh), o (sigmoid)
                nc.scalar.activation(out=ifgo[:, 2, :], in_=hh[:, 0:HD],
                                     func=AF.Tanh)
                nc.scalar.activation(out=ifgo[:, 3, :], in_=hh[:, HD:2 * HD],
                                     func=AF.Sigmoid)
        ksl = slice(k * HD, (k + 1) * HD)
        tm1 = p3.tile([P, HD], F32, tag="tm1", bufs=4, name=f"tm1_{k}_{cb}")
        nc.vector.tensor_mul(tm1, ifgo[:, 1, :], cxt[:, ksl])
        tm2 = p3.tile([P, HD], BF16, tag="tm2", bufs=4, name=f"tm2_{k}_{cb}")
        nc.gpsimd.tensor_mul(tm2, ifgo[:, 0, :], ifgo[:, 2, :])
        ck = p3.tile([P, HD], F32, tag="ck", bufs=4, name=f"ck{k}_{cb}")
        nc.vector.tensor_add(ck, tm1, tm2)
        tck = p3.tile([P, HD], BF16, tag="tck", bufs=4, name=f"tck{k}_{cb}")
        nc.scalar.activation(out=tck, in_=ck, func=AF.Tanh)
        nc.vector.tensor_mul(h_new[cb][:, ksl], ifgo[:, 3, :], tck)
        # blend c: overwrite active blocks of cx staging with c_new
        nc.vector.copy_predicated(out=cxt[:, ksl],
                                  mask=masku8_t[cb][:, k, :], data=ck)

    def attention(cb):
        # h_new^T  (16x PE transpose, packed 4-up in PSUM)
        hT = p2.tile([P, 16, P], BF16, tag="hT", name=f"hT{cb}")
        for grp in range(4):
            tp = psT.tile([P, 4, P], BF16, tag="tp", bufs=1,
                          name=f"tpa{cb}_{grp}")
            for j in range(4):
                nc.tensor.transpose(tp[:, j, :],
                                    h_new[cb][:, (4 * grp + j) * P:
                                              (4 * grp + j + 1) * P], identB)
            nc.vector.tensor_copy(out=hT[:, 4 * grp:4 * grp + 4, :], in_=tp)
        # mha projections: qkv[b, k, 192]
        qkv = p2.tile([P, NK, 3 * EM], BF16, tag="qkv", name=f"qkv{cb}")
        for k in range(NK):
            qp = psA.tile([P, 3 * EM], F32, tag="att", name=f"qp{cb}_{k}")
            for kc in range(2):
                nc.tensor.matmul(qp, hT[:, 2 * k + kc, :],
                                 wmha_t[:, kc, k, :],
                                 start=(kc == 0), stop=(kc == 1))
            nc.scalar.copy(out=qkv[:, k, :], in_=qp)
        qm = qkv[:, :, 0:EM].rearrange("p k (h e) -> p k h e", e=DKM)
        km = qkv[:, :, EM:2 * EM].rearrange("p k (h e) -> p k h e", e=DKM)
        vm = qkv[:, :, 2 * EM:3 * EM].rearrange("p k (h e) -> p k h e", e=DKM)
        # vmP[h, e, k] for unit-stride o-product
        vmP = p2.tile([P, NH, DKM, NK], BF16, tag="vmP", name=f"vmP{cb}")
        nc.scalar.copy(out=vmP, in_=vm.transpose([0, 2, 3, 1]))
        o_t = p2.tile([P, NK, NH * DKM], F32, tag="o", name=f"o{cb}")
        sc = p2.tile([P, NK, NH, NK], F32, tag="sc", name=f"sc{cb}")
        # per-head independent chains (<=3 free dims per AP; pipelines DVE)
        for h in range(NH):
            prod = p2.tile([P, NK, NK, DKM], BF16, tag="prod",
                           name=f"prod{cb}_{h}")
            nc.vector.tensor_mul(
                prod,
                qm[:, :, h, :].unsqueeze(2).broadcast_to([P, NK, NK, DKM]),
                km[:, :, h, :].unsqueeze(1).broadcast_to([P, NK, NK, DKM]))
            nc.vector.tensor_reduce(out=sc[:, :, h, :], in_=prod,
                                    axis=AX.X, op=ALU.add)
        esc = p2.tile([P, NK, NH, NK], BF16, tag="esc", name=f"esc{cb}")
        nc.scalar.activation(out=esc, in_=sc, func=AF.Exp, scale=0.25)
        esum = p2.tile([P, NK, NH], F32, tag="esum", name=f"esum{cb}")
        nc.vector.tensor_reduce(out=esum, in_=esc, axis=AX.X, op=ALU.add)
        recip = p2.tile([P, NK, NH], F32, tag="recip", name=f"recip{cb}")
        nc.vector.reciprocal(out=recip, in_=esum)
        a_t = p2.tile([P, NK, NH, NK], BF16, tag="a", name=f"a{cb}")
        nc.vector.tensor_mul(a_t, esc, recip.unsqueeze(3)
                             .broadcast_to([P, NK, NH, NK]))
        for h in range(NH):
            prod2 = p2.tile([P, NK, DKM, NK], BF16, tag="prod",
                            name=f"prod2_{cb}_{h}")
            nc.vector.tensor_mul(
                prod2,
                a_t[:, :, h, :].unsqueeze(2).broadcast_to([P, NK, DKM, NK]),
                vmP[:, h].unsqueeze(1).broadcast_to([P, NK, DKM, NK]))
            nc.vector.tensor_reduce(
                out=o_t[:, :, h * DKM:(h + 1) * DKM],
                in_=prod2, axis=AX.X, op=ALU.add)
        # oT via PE transpose (f32), packed 4-up
        oT = p2.tile([EM, NK, P], BF16, tag="oT", name=f"oT{cb}")
        for grp in range(2):
            tp = psT.tile([EM, 4, P], F32, tag="tp", bufs=1,
                          name=f"tpo{cb}_{grp}")
            for j in range(4):
                q = 4 * grp + j
                nc.tensor.transpose(tp[:, j, :], o_t[:, q, :], identF)
            nc.vector.tensor_copy(out=oT[:, 4 * grp:4 * grp + 4, :], in_=tp)
        # fc/gate + residual gating
        att = p2.tile([P, NK, HD], BF16, tag="att_all", name=f"att{cb}")
        for q in range(NK):
            fg = psA.tile([P, 2 * HD], F32, tag="att", name=f"fg{cb}_{q}")
            nc.tensor.matmul(fg, oT[:, q, :], wfg_t,
                             start=True, stop=not has_bias2)
            if has_bias2:
                nc.tensor.matmul(fg, onesF[0:1, 0:P].bitcast(F32R),
                                 biasfg_t.bitcast(F32R),
                                 start=False, stop=True)
            af = p3.tile([P, HD], BF16, tag="af", name=f"af{cb}_{q}")
            nc.scalar.activation(out=af, in_=fg[:, 0:HD], func=AF.Tanh)
            ag = p3.tile([P, HD], BF16, tag="ag", name=f"ag{cb}_{q}")
            nc.scalar.activation(out=ag, in_=fg[:, HD:2 * HD], func=AF.Sigmoid)
            nc.vector.tensor_mul(att[:, q, :], ag, af)
        hf = p2.tile([P, NK * HD], BF16, tag="hf", name=f"hf{cb}")
        nc.vector.tensor_add(hf, h_new[cb],
                             att.rearrange("p q e -> p (q e)"))
        # hx blend + mask output
        hx_t = p2.tile([P, NK * HD], F32, tag="hx", name=f"hx{cb}")
        nc.sync.dma_start(out=hx_t, in_=din["hx"][cb * P:(cb + 1) * P, :])
        nc.vector.copy_predicated(out=hx_t, mask=masku8_t[cb]
                                  .rearrange("p k e -> p (k e)"), data=hf)
        nc.sync.dma_start(out=hx_out[cb * P:(cb + 1) * P, :], in_=hx_t)
        maskF = p1.tile([P, NK, HD], F32, tag="maskF", name=f"mF{cb}")
        nc.gpsimd.tensor_copy(out=maskF, in_=mblk_t[cb].unsqueeze(2)
                              .broadcast_to([P, NK, HD]))
        nc.sync.dma_start(out=mask_out[cb * P:(cb + 1) * P, :],
                          in_=maskF.rearrange("p k e -> p (k e)"))

    # pass loop over cb pairs: attention(pair i) overlaps gates(pair i+1)
    passes = PASSES_OVERRIDE if (PASSES_OVERRIDE and ncb == 4) else [
        list(range(i, min(i + 2, ncb))) for i in range(0, ncb, 2)]
    for pi, cbs in enumerate(passes):
        cxs = {}

        def load_cx():
            for cb in cbs:
                t = p2.tile([P, NK * HD], F32, tag="cx",
                            bufs=max(len(c) for c in passes) + 1,
                            name=f"cx{cb}")
                nc.sync.dma_start(out=t,
                                  in_=din["cx"][cb * P:(cb + 1) * P, :])
                cxs[cb] = t
        for k in range(NK):
            wc_h, wh_h = [], []
            for half in range(2):
                gsl = slice(half * 512, (half + 1) * 512)
                wc = p2.tile([P, 4, 512], F8, tag="wcomb", bufs=4,
                             name=f"wc{pi}_{k}_{half}")
                nc.sync.dma_start(out=wc, in_=din["wcomb"][k][:, gsl]
                                  .rearrange("(c p) g -> p c g", p=P))
                wc_h.append(wc)
                if k < 4:
                    if pi == 0 and half == 0:
                        nc.sync.dma_start(out=wh_res[:, k], in_=din["whhT"][k]
                                          .rearrange("(c p) g -> p c g", p=P))
                    wh_h.append(wh_res[:, k, :, gsl])
                else:
                    wh = p2.tile([P, 2, 512], BF16, tag="whh", bufs=4,
                                 name=f"wh{pi}_{k}_{half}")
                    nc.sync.dma_start(out=wh, in_=din["whhT"][k][:, gsl]
                                      .rearrange("(c p) g -> p c g", p=P))
                    wh_h.append(wh)
            if pi == 0 and k > 0:
                nc.sync.dma_start(out=hxT16[:, 2 * k:2 * k + 2, :],
                                  in_=hxT16_r[:, 2 * k:2 * k + 2, :])
            if k == 0:
                load_cx()
            for cb in cbs:
                gates_lstm(k, cb, cxs[cb], wc_h, wh_h)
        for cb in cbs:
            nc.sync.dma_start(out=cx_out[cb * P:(cb + 1) * P, :], in_=cxs[cb])
        if pi == 0:
            load_wmha()
        for cb in cbs:
            attention(cb)
    ctx.close()


# ---------------------------------------------------------------------------
# host side
# ---------------------------------------------------------------------------

_CACHE = {}


def _get_program(bpc, has_bias, has_bias2):
    key = (bpc, has_bias, has_bias2)
    if key not in _CACHE:
        _CACHE[key] = _build_program(bpc, has_bias, has_bias2)
    return _CACHE[key]


def _host_prep(inputs, ncores=NCORES):
    f32 = np.float32
    inp = np.ascontiguousarray(np.asarray(inputs["inp"], dtype=f32))
    hx = np.ascontiguousarray(np.asarray(inputs["hx"], dtype=f32))
    cx = np.ascontiguousarray(np.asarray(inputs["cx"], dtype=f32))
    B = inp.shape[0]
    bpc = B // ncores

    Wv1 = np.asarray(inputs["Wv_i"][1], dtype=f32)          # (C, 4*HD... ATT_OUT)
    Wih = np.asarray(inputs["Wih"], dtype=f32)              # (NK, GD, ATT_OUT)
    wcomb = np.einsum("cd,kgd->kcg", Wv1.astype(np.float64),
                      Wih.astype(np.float64)).astype(f32).astype(
                          ml_dtypes.float8_e4m3)
    whhT = np.ascontiguousarray(
        np.asarray(inputs["Whh"], dtype=f32).transpose(0, 2, 1)).astype(BF)
    # host score path (fp32, must match reference ranking exactly)
    wqi = np.asarray(inputs["Wq_i"], dtype=f32)
    wk1 = np.asarray(inputs["Wk_i"][1], dtype=f32)
    k1_h = inp @ wk1
    q_h = np.einsum("bkd,kde->bke", hx.reshape(B, NK, HD), wqi)
    s_h = np.einsum("bke,be->bk", q_h, k1_h)
    sig_h = (1.0 / (1.0 + np.exp(-s_h.astype(np.float64) / 8.0))).astype(f32)
    thr_h = np.sort(s_h, axis=1)[:, NK - 4:NK - 3]
    mblk_h = (s_h >= thr_h).astype(f32)
    wmha = np.concatenate([np.asarray(inputs["Wq_m"], dtype=f32),
                           np.asarray(inputs["Wk_m"], dtype=f32),
                           np.asarray(inputs["Wv_m"], dtype=f32)],
                          axis=2).astype(BF)
    wfg = np.concatenate([np.asarray(inputs["fc_w"], dtype=f32).T,
                          np.asarray(inputs["gate_w"], dtype=f32).T],
                         axis=1).astype(BF)                  # (EM, 2*HD)
    biasg = (np.asarray(inputs["b_ih"], dtype=f32)
             + np.asarray(inputs["b_hh"], dtype=f32))        # (NK, GD)
    biasfg = np.concatenate([np.asarray(inputs["fc_b"], dtype=f32),
                             np.asarray(inputs["gate_b"], dtype=f32)])[None, :]
    has_bias = bool(np.any(biasg))
    has_bias2 = bool(np.any(biasfg))

    in_maps = []
    for m in range(ncores):
        sl = slice(m * bpc, (m + 1) * bpc)
        d = dict(
            hxT16=np.ascontiguousarray(hx[sl].T).astype(BF),
            inp16b=inp[sl].astype(BF),
            sig=sig_h[sl], mblk=mblk_h[sl],
            mblk8=mblk_h[sl].astype(np.uint8),
            hx=inp_cont(hx[sl]),
            cx=inp_cont(cx[sl]),
            wcomb=wcomb, whhT=whhT, wmha=wmha, wfg=wfg,
        )
        if has_bias:
            d["biasg"] = biasg
        if has_bias2:
            d["biasfg"] = biasfg
        in_maps.append(d)
    return in_maps, bpc, has_bias, has_bias2


def inp_cont(x):
    return np.ascontiguousarray(x)


def run(inputs, trace=False, **kw):
    in_maps, bpc, has_bias, has_bias2 = _host_prep(inputs)
    nc = _get_program(bpc, has_bias, has_bias2)
    res = run_bass_kernel_spmd(nc, in_maps, core_ids=list(range(NCORES)),
                               trace=trace, **kw)
    hx_out = np.concatenate([r["hx_out"] for r in res.results], axis=0)
    cx_out = np.concatenate([r["cx_out"] for r in res.results], axis=0)
    mask = np.concatenate([r["mask"] for r in res.results], axis=0)
    return (hx_out, cx_out, mask), res


def kernel(**inputs):
    out, _ = run(inputs)
    return out



# revision 3
# speedup vs baseline: 5.8922x; 5.8922x over previous
"""Trainium2 Bass kernel for nn_BlocksCore (moe_routing) — v2.

Contract: kernel(**inputs) takes FULL unsharded inputs (inp (4096,512),
hx/cx (4096,2048), weights, step) and returns (hx_out, cx_out, mask) each
(4096, 2048) f32, matching reference._fwd.

v2 vs v1: weights resident in SBUF (streamed once, not 3x), fp8 DoubleRow
for the wcomb gates matmuls, fat-rhs scaled-transpose (one matmul per
contraction chunk produces all 8 blocks' scaled inpT), DMA-engine xbar
transposes for hxT/hT/oT (replaces PE transpose + DVE copy chains), bf16
hx/cx/outputs (host upcasts), mask assembled on host, gate order permuted
to (i,f,o,g) so one 768-wide sigmoid covers i,f,o, per-k-pair batched cell
ops, q-pair-batched fc/gate matmuls via block-diagonal weights.

Sharding: pure data parallel over 8 cores (512 batch rows each). Host
computes the tiny score/top-k path (fp32-exact ranking) and all dtype casts.
"""
import os
import sys

import numpy as np

try:
    import concourse.bass as bass
except ImportError:  # container puts the repo here
    for _p in ("/opt/trn_rl_repo", "/root/.axon_site/_ro/trn_rl_repo"):
        if os.path.isdir(_p) and _p not in sys.path:
            sys.path.insert(0, _p)
    import concourse.bass as bass

import ml_dtypes
import concourse.bacc as bacc
import concourse.mybir as mybir
import concourse.tile as tile
from concourse.bass_utils import run_bass_kernel_spmd
from concourse.masks import make_identity

F32 = mybir.dt.float32
F32R = mybir.dt.float32r
BF16 = mybir.dt.bfloat16
F8 = mybir.dt.float8e4
U8 = mybir.dt.uint8
AF = mybir.ActivationFunctionType
ALU = mybir.AluOpType
AX = mybir.AxisListType
DR = mybir.MatmulPerfMode.DoubleRow
BF = ml_dtypes.bfloat16

NCORES = 8
P = 128          # partition rows per batch chunk
NK = 8           # blocks
HD = 256         # block size (BS)
GD = 1024        # gates per block (4*HD)
C = 512          # NINP
NH, DKM = 4, 16  # mha heads, head dim
EM = NH * DKM    # 64


def _build_program(bpc, has_bias, has_bias2):
    ncb = bpc // P
    nc = bacc.Bacc("TRN2", target_bir_lowering=False, debug=False,
                   num_devices=NCORES)

    din = {}
    def dram_in(name, shape, dtype=F32):
        din[name] = nc.dram_tensor(name, list(shape), dtype,
                                   kind="ExternalInput").ap()
        return din[name]

    dram_in("iuT8", (bpc // P, P, 4, NK, P), F8)
    dram_in("hx16b", (bpc, NK * HD), BF16)
    dram_in("cx16b", (bpc, NK * HD), BF16)
    dram_in("mblk8", (bpc, NK), U8)
    dram_in("wcomb", (NK, C, GD), F8)          # gate order i,f,o,g
    dram_in("whhT", (NK, HD, GD), BF16)        # gate order i,f,o,g
    dram_in("wmha", (NK, HD, 3 * EM), BF16)
    dram_in("wfg2", (P, 2, 2 * HD), BF16)      # block-diag (q-pair)
    if has_bias:
        dram_in("biasg", (NK, GD))             # gate order i,f,o,g
    if has_bias2:
        dram_in("biasfg", (1, 2 * HD))

    hx_out = nc.dram_tensor("hx_out16", [bpc, NK * HD], BF16,
                            kind="ExternalOutput").ap()
    cx_out = nc.dram_tensor("cx_out16", [bpc, NK * HD], BF16,
                            kind="ExternalOutput").ap()

    with tile.TileContext(nc) as tc:
        _emit(tc, din, hx_out, cx_out, ncb, has_bias, has_bias2)
    nc.compile()
    return nc


def _emit(tc, din, hx_out, cx_out, ncb, has_bias, has_bias2):
    nc = tc.nc
    import contextlib
    ctx = contextlib.ExitStack()
    p1 = ctx.enter_context(tc.tile_pool(name="p1", bufs=1))
    p2 = ctx.enter_context(tc.tile_pool(name="p2", bufs=2))
    p3 = ctx.enter_context(tc.tile_pool(name="p3", bufs=3))
    p4 = ctx.enter_context(tc.tile_pool(name="p4", bufs=4))
    psG = ctx.enter_context(tc.tile_pool(name="psG", bufs=2, space="PSUM"))
    psA = ctx.enter_context(tc.tile_pool(name="psA", bufs=2, space="PSUM"))

    # ---------------- static loads ----------------
    # per-cb activations on the SP queue; the 8MB weight stream on the
    # Activation HWDGE queue so it never blocks them.
    wfg2_t = p1.tile([P, 2, 2 * HD], BF16, tag="wfg2")
    nc.sync.dma_start(out=wfg2_t, in_=din["wfg2"])
    mblk8_all = p1.tile([P, ncb, NK], U8, tag="mblk8_all")
    nc.sync.dma_start(out=mblk8_all,
                      in_=din["mblk8"].rearrange("(cb p) k -> p cb k", p=P))
    cx16_t, hxT_t, iuT_t = {}, {}, {}

    def emit_loads(cb):
        bsl = slice(cb * P, (cb + 1) * P)
        cx16 = p2.tile([P, NK * HD], BF16, tag="cx16", name=f"cx16_{cb}")
        nc.sync.dma_start(out=cx16, in_=din["cx16b"][bsl])
        hxT = p2.tile([P, 16, P], BF16, tag="hxT", name=f"hxT_{cb}")
        nc.sync.dma_start_transpose(hxT, din["hx16b"][bsl])
        iuT = p2.tile([P, 4, NK, P], F8, tag="iuT", name=f"iuT_{cb}")
        nc.sync.dma_start(out=iuT, in_=din["iuT8"][cb])
        cx16_t[cb] = cx16
        hxT_t[cb] = hxT
        iuT_t[cb] = iuT

    emit_loads(0)
    emit_loads(1)

    # resident weights as per-k tiles (separate DMAs -> per-k readiness),
    # k-ascending on the ACT queue
    wc_p, wh_p = [], []
    for kq in range(NK // 2):
        wh = p1.tile([P, 2, 2, GD], BF16, tag=f"whh{kq}", name=f"wh_{kq}")
        nc.scalar.dma_start(out=wh, in_=din["whhT"][2 * kq:2 * kq + 2]
                            .rearrange("k (c p) g -> p k c g", p=P))
        wc = p1.tile([P, 2, 4, GD], F8, tag=f"wcomb{kq}", name=f"wc_{kq}")
        nc.scalar.dma_start(out=wc, in_=din["wcomb"][2 * kq:2 * kq + 2]
                            .rearrange("k (c p) g -> p k c g", p=P))
        wh_p.append(wh)
        wc_p.append(wc)
    wh_t = [wh_p[k // 2][:, k % 2] for k in range(NK)]
    wc_t = [wc_p[k // 2][:, k % 2] for k in range(NK)]
    wmha_t = p1.tile([P, NK, 2, 3 * EM], BF16, tag="wmha")
    nc.scalar.dma_start(out=wmha_t, in_=din["wmha"].rearrange(
        "k (c p) e -> p k c e", p=P))
    if has_bias:
        biasg_t = p1.tile([1, NK, GD], F32, tag="biasg")
        nc.scalar.dma_start(out=biasg_t, in_=din["biasg"].unsqueeze(0))
    if has_bias2:
        biasfg_t = p1.tile([1, 2 * HD], F32, tag="biasfg")
        nc.scalar.dma_start(out=biasfg_t, in_=din["biasfg"])
    if has_bias or has_bias2:
        onesF = p1.tile([1, P], F32, tag="onesF")
        nc.vector.memset(onesF, 1.0)

    # ---------------- gates, pair-major (paced by the weight stream) -----
    # All-tanh formulation in a x2 domain: sigma(x) = (tanh(x/2)+1)/2.
    # hx16/cx16 arrive pre-doubled, whhT/wmha pre-halved, outputs are
    # halved on the host.  ck2 = 2*c_new, hnew holds 2*h_new.
    hnew_t = {}
    hTh_t = {}
    ifo_p = {}

    def gates_pair(pair):
        cbs = (2 * pair, 2 * pair + 1)
        for cb in cbs:
            hnew_t[cb] = p4.tile([P, NK * HD], BF16, tag="hnew",
                                 name=f"hnew_{cb}")
            hTh_t[cb] = [None, None]
            ifo_p[cb] = []
        for k in range(NK):
            for cb in cbs:
                hxT, cx16, iuT = hxT_t[cb], cx16_t[cb], iuT_t[cb]
                hnew = hnew_t[cb]
                hh = psG.tile([P, GD], F32, tag="hh", name=f"hh_{cb}_{k}")
                for half in range(2):
                    gsl = slice(half * 512, (half + 1) * 512)
                    nc.tensor.matmul(hh[:, gsl], hxT[:, 2 * k, :],
                                     wh_t[k][:, 0, gsl],
                                     start=True, stop=False)
                    nc.tensor.matmul(hh[:, gsl], hxT[:, 2 * k + 1, :],
                                     wh_t[k][:, 1, gsl],
                                     start=False, stop=False)
                if has_bias:
                    nc.tensor.matmul(hh, onesF[0:1, 0:P].bitcast(F32R),
                                     biasg_t[0:1, k, :].bitcast(F32R),
                                     start=False, stop=False)
                for j in range(2):
                    for half in range(2):
                        gsl = slice(half * 512, (half + 1) * 512)
                        nc.tensor.matmul(
                            hh[:, gsl], iuT[:, 2 * j:2 * j + 2, k, :],
                            wc_t[k][:, 2 * j:2 * j + 2, gsl],
                            start=False, stop=(j == 1), perf_mode=DR)
                if k % 2 == 0:
                    ifo2 = p3.tile([P, 2, 4, HD], BF16, tag="ifo", bufs=4,
                                   name=f"ifo_{cb}_{k // 2}")
                    ifo_p[cb].append(ifo2)
                ifo2 = ifo_p[cb][-1]
                # all four gates in one tanh(x/2): g-columns are pre-doubled
                # on the host so tanh(0.5 * 2g) = tanh(g)
                nc.scalar.activation(out=ifo2[:, k % 2], in_=hh.rearrange(
                    "p (a e) -> p a e", a=4), func=AF.Tanh, scale=0.5)
                if k % 2 == 1:
                    kp = k // 2
                    ksl2 = slice(kp * 2 * HD, (kp + 1) * 2 * HD)
                    cxp = cx16[:, ksl2].rearrange("p (a e) -> p a e", a=2)
                    # tm1 = (tf+1)*cx2 = 4*sig(f)*c ; tm2 = (ti+1)*tg
                    tm1 = p3.tile([P, 2, HD], BF16, tag="tm1", bufs=2,
                                  name=f"tm1_{cb}_{kp}")
                    nc.vector.scalar_tensor_tensor(
                        out=tm1, in0=ifo2[:, :, 1, :], scalar=1.0, in1=cxp,
                        op0=ALU.add, op1=ALU.mult)
                    tm2 = p3.tile([P, 2, HD], BF16, tag="tm2", bufs=2,
                                  name=f"tm2_{cb}_{kp}")
                    nc.vector.scalar_tensor_tensor(
                        out=tm2, in0=ifo2[:, :, 0, :], scalar=1.0,
                        in1=ifo2[:, :, 3, :], op0=ALU.add, op1=ALU.mult)
                    # ck2 = 0.5*tm1 + tm2 = 2*c_new
                    ck2 = p3.tile([P, 2, HD], BF16, tag="ck2", bufs=2,
                                  name=f"ck2_{cb}_{kp}")
                    nc.vector.scalar_tensor_tensor(
                        out=ck2, in0=tm1, scalar=0.5, in1=tm2,
                        op0=ALU.mult, op1=ALU.add)
                    tck = p3.tile([P, 2, HD], BF16, tag="tck", bufs=2,
                                  name=f"tck_{cb}_{kp}")
                    nc.scalar.activation(out=tck, in_=ck2, func=AF.Tanh,
                                         scale=0.5)
                    # hnew = (to+1)*tanh(c_new) = 2*h_new
                    nc.vector.scalar_tensor_tensor(
                        out=hnew[:, ksl2].rearrange("p (a e) -> p a e", a=2),
                        in0=ifo2[:, :, 2, :], scalar=1.0, in1=tck,
                        op0=ALU.add, op1=ALU.mult)
                    # blend c in place (2x domain): frozen blocks keep cx2
                    nc.vector.copy_predicated(
                        out=cxp,
                        mask=mblk8_all[:, cb, 2 * kp:2 * kp + 2]
                        .unsqueeze(2).broadcast_to([P, 2, HD]),
                        data=ck2)
                    if k == 3 or k == 7:
                        half_i = k // 4
                        hT = p2.tile([P, NK, P], BF16, tag=f"hTh{half_i}",
                                     name=f"hT_{cb}_{half_i}")
                        nc.sync.dma_start_transpose(
                            hT, hnew[:, half_i * 1024:(half_i + 1) * 1024])
                        hTh_t[cb][half_i] = hT
                    if k == NK - 1:
                        nc.sync.dma_start(
                            out=cx_out[cb * P:(cb + 1) * P, :], in_=cx16)

    # ---------------- attention, split front (DVE chain) / back ----------
    st = {}

    def attn_front(cb):
        hnew, hTh = hnew_t[cb], hTh_t[cb]
        hx16 = p2.tile([P, NK * HD], BF16, tag="hx16", name=f"hx16_{cb}")
        nc.sync.dma_start(out=hx16, in_=din["hx16b"][cb * P:(cb + 1) * P])
        qkv = p2.tile([P, NK, 3 * EM], BF16, tag="qkv", name=f"qkv_{cb}")
        for kk in range(2):
            # pad to 256/slice so no matmul output straddles a PSUM bank
            qp = psA.tile([P, 4, 4 * EM], F32, tag="aux",
                          name=f"qp_{cb}_{kk}")
            for i in range(4):
                nc.tensor.matmul(qp[:, i, 0:3 * EM], hTh[kk][:, 2 * i, :],
                                 wmha_t[:, 4 * kk + i, 0, :],
                                 start=True, stop=False)
                nc.tensor.matmul(qp[:, i, 0:3 * EM], hTh[kk][:, 2 * i + 1, :],
                                 wmha_t[:, 4 * kk + i, 1, :],
                                 start=False, stop=True)
            if kk == 0:
                nc.scalar.copy(out=qkv[:, 0:4, :], in_=qp[:, :, 0:3 * EM])
            else:
                nc.scalar.copy(out=qkv[:, 4:8, :], in_=qp[:, :, 0:3 * EM])
        qm = qkv[:, :, 0:EM].rearrange("p k (h e) -> p k h e", e=DKM)
        km = qkv[:, :, EM:2 * EM].rearrange("p k (h e) -> p k h e", e=DKM)
        vm = qkv[:, :, 2 * EM:3 * EM].rearrange("p k (h e) -> p k h e",
                                                e=DKM)
        vmP = p2.tile([P, NH, DKM, NK], BF16, tag="vmP", name=f"vmP_{cb}")
        nc.scalar.copy(out=vmP, in_=vm.transpose([0, 2, 3, 1]))
        sc = p2.tile([P, NH, NK, NK], F32, tag="sc", name=f"sc_{cb}")
        with nc.allow_low_precision(reason="scores tiny; softmax "
                                    "insensitive"):
            for h in range(NH):
                eng = nc.gpsimd if h >= 2 else nc.vector
                prod = p3.tile([P, NK, NK, DKM], BF16, tag="prod", bufs=2,
                               name=f"prod_{cb}_{h}")
                eng.tensor_mul(
                    prod,
                    qm[:, :, h, :].unsqueeze(2).broadcast_to(
                        [P, NK, NK, DKM]),
                    km[:, :, h, :].unsqueeze(1).broadcast_to(
                        [P, NK, NK, DKM]))
                f1 = p3.tile([P, NK, NK, DKM // 2], BF16, tag="f1", bufs=2,
                             name=f"f1_{cb}_{h}")
                eng.tensor_add(f1, prod[:, :, :, 0:8], prod[:, :, :, 8:16])
                nc.vector.tensor_reduce(out=sc[:, h], in_=f1, axis=AX.X,
                                        op=ALU.add)
        esc = p2.tile([P, NH, NK, NK], BF16, tag="esc", name=f"esc_{cb}")
        esum = p2.tile([P, NH, NK], F32, tag="esum", name=f"esum_{cb}")
        recip = p2.tile([P, NH, NK], F32, tag="recip", name=f"recip_{cb}")
        a_t = p2.tile([P, NH, NK, NK], BF16, tag="a_t", name=f"a_{cb}")
        for hh2 in range(2):
            hs = slice(2 * hh2, 2 * hh2 + 2)
            nc.scalar.activation(out=esc[:, hs], in_=sc[:, hs], func=AF.Exp,
                                 scale=0.25)
            nc.vector.tensor_reduce(out=esum[:, hs], in_=esc[:, hs],
                                    axis=AX.X, op=ALU.add)
            nc.vector.reciprocal(out=recip[:, hs], in_=esum[:, hs])
            nc.vector.tensor_mul(a_t[:, hs], esc[:, hs],
                                 recip[:, hs].unsqueeze(3)
                                 .broadcast_to([P, 2, NK, NK]))
        o16 = p2.tile([P, NK, NH, DKM], BF16, tag="o16", name=f"o16_{cb}")
        with nc.allow_low_precision(reason="attention output bf16"):
            for h in range(NH):
                eng = nc.gpsimd if h >= 2 else nc.vector
                prod2 = p3.tile([P, NK, DKM, NK], BF16, tag="prod", bufs=2,
                                name=f"prod2_{cb}_{h}")
                eng.tensor_mul(
                    prod2,
                    a_t[:, h].unsqueeze(2).broadcast_to([P, NK, DKM, NK]),
                    vmP[:, h].unsqueeze(1).broadcast_to([P, NK, DKM, NK]))
                f2 = p3.tile([P, NK, DKM, NK // 2], BF16, tag="f1", bufs=2,
                             name=f"f2_{cb}_{h}")
                eng.tensor_add(f2, prod2[:, :, :, 0:4], prod2[:, :, :, 4:8])
                nc.vector.tensor_reduce(out=o16[:, :, h, :], in_=f2,
                                        axis=AX.X, op=ALU.add)
        oT2 = p2.tile([P, NK // 2, P], BF16, tag="oT2", name=f"oT2_{cb}")
        nc.sync.dma_start_transpose(oT2,
                                    o16.rearrange("p q h e -> p (q h e)"))
        st[cb] = (hx16, oT2)

    def attn_back(cb):
        hnew = hnew_t[cb]
        hx16, oT2 = st.pop(cb)
        for j in range(4):
            fgp = psA.tile([P, 2, 2 * HD], F32, tag="aux",
                           name=f"fgp_{cb}_{j}")
            for qq in range(2):
                nc.tensor.matmul(fgp[:, qq, :], oT2[:, j, :],
                                 wfg2_t[:, qq, :],
                                 start=True, stop=not has_bias2)
            if has_bias2:
                fgf = fgp.rearrange("p a b -> p (a b)")
                nc.tensor.matmul(fgf[:, 0:2 * HD],
                                 onesF[0:1, 0:P].bitcast(F32R),
                                 biasfg_t.bitcast(F32R),
                                 start=False, stop=False)
                nc.tensor.matmul(fgf[:, 2 * HD:4 * HD],
                                 onesF[0:1, 0:P].bitcast(F32R),
                                 biasfg_t.bitcast(F32R),
                                 start=False, stop=True)
            # one tanh(x/2) for both: fc columns pre-doubled on the host
            fga = p3.tile([P, 2, 2, HD], BF16, tag="fga", bufs=2,
                          name=f"fga_{cb}_{j}")
            nc.scalar.activation(out=fga, in_=fgp.rearrange(
                "p a (b e) -> p a b e", b=2), func=AF.Tanh, scale=0.5)
            # u = (ag+1)*af = 2*att ; hf = u + hnew = 2*(h_new+att)
            u = p3.tile([P, 2, HD], BF16, tag="u", bufs=2,
                        name=f"u_{cb}_{j}")
            nc.vector.scalar_tensor_tensor(
                out=u, in0=fga[:, :, 1, :], scalar=1.0, in1=fga[:, :, 0, :],
                op0=ALU.add, op1=ALU.mult)
            hfp = p3.tile([P, 2, HD], BF16, tag="hfp", bufs=2,
                          name=f"hfp_{cb}_{j}")
            nc.vector.tensor_add(
                hfp, u, hnew[:, 2 * j * HD:(2 * j + 2) * HD].rearrange(
                    "p (a e) -> p a e", a=2))
            nc.vector.copy_predicated(
                out=hx16[:, 2 * j * HD:(2 * j + 2) * HD].rearrange(
                    "p (a e) -> p a e", a=2),
                mask=mblk8_all[:, cb, 2 * j:2 * j + 2]
                .unsqueeze(2).broadcast_to([P, 2, HD]),
                data=hfp)
        nc.sync.dma_start(out=hx_out[cb * P:(cb + 1) * P, :], in_=hx16)

    gates_pair(0)
    attn_front(0)
    emit_loads(2)
    emit_loads(3)
    attn_front(1)
    gates_pair(1)
    attn_back(0)
    attn_front(2)
    attn_back(1)
    attn_front(3)
    attn_back(2)
    attn_back(3)
    ctx.close()


# ---------------------------------------------------------------------------
# host side
# ---------------------------------------------------------------------------

_CACHE = {}
# gate order permutation: reference (i,f,g,o) -> device (i,f,o,g)
_GPERM = np.concatenate([np.arange(0, 512),          # i, f
                         np.arange(768, 1024),       # o
                         np.arange(512, 768)])       # g


def _get_program(bpc, has_bias, has_bias2):
    key = (bpc, has_bias, has_bias2)
    if key not in _CACHE:
        _CACHE[key] = _build_program(bpc, has_bias, has_bias2)
    return _CACHE[key]


def _host_prep(inputs, ncores=NCORES):
    f32 = np.float32
    inp = np.ascontiguousarray(np.asarray(inputs["inp"], dtype=f32))
    hx = np.ascontiguousarray(np.asarray(inputs["hx"], dtype=f32))
    cx = np.ascontiguousarray(np.asarray(inputs["cx"], dtype=f32))
    B = inp.shape[0]
    bpc = B // ncores

    Wv1 = np.asarray(inputs["Wv_i"][1], dtype=f32)          # (C, ATT_OUT)
    Wih = np.asarray(inputs["Wih"], dtype=f32)              # (NK, GD, ATT_OUT)
    wcomb = np.einsum("cd,kgd->kcg", Wv1.astype(np.float64),
                      Wih.astype(np.float64)).astype(f32)
    gscale8 = np.ones((GD,), f32)
    gscale8[3 * HD:] = 2.0
    wcomb = np.ascontiguousarray(wcomb[:, :, _GPERM] * gscale8).astype(
        ml_dtypes.float8_e4m3)
    # x2-domain: hx/cx shipped doubled; whhT/wmha halved to compensate.
    # g-gate columns doubled so one tanh(x/2) covers all four gates.
    gscale = np.ones((GD,), f32)
    gscale[3 * HD:] = 2.0
    whhT = np.asarray(inputs["Whh"], dtype=f32).transpose(0, 2, 1)
    whhT = np.ascontiguousarray(0.5 * whhT[:, :, _GPERM] * gscale).astype(BF)
    # host score path (fp32, must match reference ranking exactly)
    wqi = np.asarray(inputs["Wq_i"], dtype=f32)
    wk1 = np.asarray(inputs["Wk_i"][1], dtype=f32)
    k1_h = inp @ wk1
    q_h = np.einsum("bkd,kde->bke", hx.reshape(B, NK, HD), wqi)
    s_h = np.einsum("bke,be->bk", q_h, k1_h)
    sig_h = (1.0 / (1.0 + np.exp(-s_h.astype(np.float64) / 8.0))).astype(f32)
    thr_h = np.sort(s_h, axis=1)[:, NK - 4:NK - 3]
    mblk_h = (s_h >= thr_h).astype(f32)
    wmha = (0.5 * np.concatenate([np.asarray(inputs["Wq_m"], dtype=f32),
                                  np.asarray(inputs["Wk_m"], dtype=f32),
                                  np.asarray(inputs["Wv_m"], dtype=f32)],
                                 axis=2)).astype(BF)
    wfgT = np.concatenate([np.asarray(inputs["fc_w"], dtype=f32).T,
                           np.asarray(inputs["gate_w"], dtype=f32).T],
                          axis=1)                          # (EM, 2*HD)
    # fc columns doubled so one tanh(x/2) yields tanh(fc) | tanh(gate/2)
    wfgT2 = wfgT.copy()
    wfgT2[:, 0:HD] *= 2.0
    wfg2 = np.zeros((P, 2, 2 * HD), dtype=f32)
    wfg2[0:EM, 0] = wfgT2
    wfg2[EM:2 * EM, 1] = wfgT2
    wfg2 = wfg2.astype(BF)
    biasg = (np.asarray(inputs["b_ih"], dtype=f32)
             + np.asarray(inputs["b_hh"], dtype=f32))[:, _GPERM]
    biasfg = np.concatenate([np.asarray(inputs["fc_b"], dtype=f32),
                             np.asarray(inputs["gate_b"], dtype=f32)])[None]
    has_bias = bool(np.any(biasg))
    has_bias2 = bool(np.any(biasfg))

    hx16 = (2.0 * hx).astype(BF)
    cx16 = (2.0 * cx).astype(BF)
    mblk8 = mblk_h.astype(np.uint8)
    # device-layout f8 scaled transposed input: iuT8[cb, p, cc, k, b] =
    # f8(bf16(inp[cb*128+b, cc*128+p]) * bf16(sig[cb*128+b, k]))
    scaled = (inp.astype(BF).astype(f32)[:, :, None]
              * sig_h.astype(BF).astype(f32)[:, None, :])   # (B, C, NK)
    iuT8 = np.ascontiguousarray(
        scaled.reshape(B // P, P, 4, P, NK).transpose(0, 3, 2, 4, 1)
    ).astype(ml_dtypes.float8_e4m3)                         # (B/P, P, 4, NK, P)

    in_maps = []
    for m in range(ncores):
        sl = slice(m * bpc, (m + 1) * bpc)
        ncb = bpc // P
        d = dict(
            iuT8=iuT8[m * ncb:(m + 1) * ncb], hx16b=hx16[sl],
            cx16b=cx16[sl], mblk8=mblk8[sl],
            wcomb=wcomb, whhT=whhT, wmha=wmha, wfg2=wfg2,
        )
        if has_bias:
            d["biasg"] = biasg
        if has_bias2:
            d["biasfg"] = biasfg
        in_maps.append(d)
    return in_maps, bpc, has_bias, has_bias2, mblk_h


def run(inputs, trace=False, **kw):
    in_maps, bpc, has_bias, has_bias2, mblk_h = _host_prep(inputs)
    nc = _get_program(bpc, has_bias, has_bias2)
    res = run_bass_kernel_spmd(nc, in_maps, core_ids=list(range(NCORES)),
                               trace=trace, **kw)
    # device works in a x2 domain; halve on upcast
    hx_out = 0.5 * np.concatenate([r["hx_out16"] for r in res.results],
                                  axis=0).astype(np.float32)
    cx_out = 0.5 * np.concatenate([r["cx_out16"] for r in res.results],
                                  axis=0).astype(np.float32)
    mask = np.repeat(mblk_h, HD, axis=1)
    return (hx_out, cx_out, mask), res


def kernel(**inputs):
    out, _ = run(inputs)
    return out
